# revision 32
# baseline (speedup 1.0000x reference)
"""Trainium2 Bass kernel for nn_Colorizer (retrieval_knn).

Computation (per reference frame r of 3, for each pixel p of a 128x128 image):
  corr[r, n, p] = <feats_t[:, p], feats_r[r, :, p + offset(n)]>   n in 13x13 window
  q_val[r, p]  = max_n corr ; q_idx[r, p] = argmax_n corr (first occurrence)
  gathered[r, c, p] = quantized_sub[r, c, p + offset(q_idx)]      (zero padded)
  out[c, p] = sum_r softmax_r(q_val)[r] * gathered[r, c, p]

Sharding: the spatial h dim is split into 8 bands of 16 rows (one per core);
each core handles all 3 refs for its band, so the softmax over refs is local
and no device collective is needed.  Host reassembles the row bands.

Device algorithm per core, per tile of 128 pixels (16 rows x 8 cols):
  - TensorE: Gram matrix between the tile's feats_t vectors (lhsT, c=128
    contraction) and the 28x20 zero-padded feats_r halo window (560 cols ->
    2 PSUM banks), then a constant additive -1e30 valid-window mask via an
    identity matmul accumulate.
  - ScalarE copies the masked PSUM to a flat SBUF tile buffer (fast PSUM
    turnaround for the matmul pipeline; SBUF reads are cheaper for the DVE).
  - DVE MAX8 + FIND_INDEX8 on the SBUF copy -> per-pixel masked max (q_val)
    and argmax index.  Per-tile (not batched) so the indirect gathers spread
    evenly over the whole kernel: each gather op costs a fixed ~1.1us of
    GPSIMD descriptor-generation time and the gather queue is the scarcest
    resource.
  - per-tile indirect DMA gathers the argmax pixel (3ch) from a per-tile
    [560, 3] DRAM table indexed by the raw window index.
  - softmax over the 3 refs without max-subtraction (corr maxima are < 90 so
    exp() cannot overflow), accumulated incrementally per ref so only a ~1us
    chain remains after the last gather.

Input DMA is priority-ordered on the sync HWDGE queue (first tiles' feats
first; feats_r of ref 0 is shipped as 3 overlapping contiguous column slabs
so the first matmul can start early), with the mask + identity on the
parallel scalar HWDGE queue.
"""

import os

import numpy as np

import concourse.bass as bass
import concourse.mybir as mybir
import concourse.tile as tile
from concourse import bacc
from concourse.bass import IndirectOffsetOnAxis
from concourse.bass_utils import run_bass_kernel_spmd

F32 = mybir.dt.float32
BF16 = mybir.dt.bfloat16
U32 = mybir.dt.uint32

NCORES = 8
NREF, C, H, W = 3, 128, 128, 128
RAD = 6                      # patch radius
PS = 2 * RAD + 1             # 13
CQ = 3                       # quantized channels
SUB = 4                      # quantized_r spatial subsample stride
ROWS = H // NCORES           # 16 rows per core
XB = 8                       # x block size
NT = W // XB                 # 16 tiles per ref
WY = ROWS + 2 * RAD          # 28 window rows
WX = XB + 2 * RAD            # 20 window cols
WIN = WY * WX                # 560
HALF = WY // 2               # 14 window rows per PSUM bank
NHALF = HALF * WX            # 280 columns per matmul
PW = W + 2 * RAD             # 140 padded width
NRT = NREF * NT              # 48 (ref, tile) pairs
NEG = -1.0e30

# ref-0 feats_r is shipped as 3 contiguous, overlapping column slabs so the
# first tiles' windows arrive early: tile t needs padded cols [8t, 8t+20).
SLABS = [(0, 28), (16, 76), (64, 140)]          # tiles 0-1 / 2-7 / 8-15
SLAB_OF_TILE = [0] * 2 + [1] * 6 + [2] * 8

_CACHE: dict = {}


def _max_index_raw(nc, out, in_max, in_values):
    """max_index accepting multi-dim / broadcast APs (e.g. duplicated needle
    views); the bass wrapper's 2-D asserts are stricter than the hardware."""
    eng = nc.vector
    return eng.add_instruction(
        mybir.InstMaxIndex(
            name=nc.get_next_instruction_name(),
            ins=[eng.lower_ap(in_max), eng.lower_ap(in_values)],
            outs=[eng.lower_ap(out)],
        )
    )


def _build_program() -> bacc.Bacc:
    nc = bacc.Bacc("TRN2", target_bir_lowering=False, debug=False)

    ft_d = nc.dram_tensor("ft", [C, ROWS * W], F32, kind="ExternalInput")
    fr0_d = [
        nc.dram_tensor(f"fr0s{s}", [C, WY * (e - b)], F32, kind="ExternalInput")
        for s, (b, e) in enumerate(SLABS)
    ]
    fr12_d = [
        nc.dram_tensor(f"fr{r}", [C, WY * PW], F32, kind="ExternalInput")
        for r in (1, 2)
    ]
    mask_d = nc.dram_tensor("mask", [128, WIN], BF16, kind="ExternalInput")
    ident_d = nc.dram_tensor("ident", [128, 128], BF16, kind="ExternalInput")
    # one [4*WIN, CQ] gather table per quad of tiles: the raw quad-local
    # argmax index is the gather row, no on-device index arithmetic needed
    qt_d = [
        nc.dram_tensor(f"qt{rt}", [WIN, CQ], F32, kind="ExternalInput")
        for rt in range(NRT)
    ]
    # raw layout [pixel_partition=(yl,xl), tile, channel]; host untangles
    out_d = nc.dram_tensor("out", [128, NT * CQ], F32, kind="ExternalOutput")

    with tile.TileContext(nc) as tc:
        with (
            tc.tile_pool(name="const", bufs=1) as constp,
            tc.tile_pool(name="psum", bufs=4, space="PSUM") as psump,
            tc.tile_pool(name="tbuf", bufs=4) as tbufp,
            tc.tile_pool(name="small", bufs=1) as smallp,
        ):
            # ---- input DMA, priority order on the sync HWDGE queue ----
            ft_sb = constp.tile([C, ROWS * W], F32, tag="ft")
            fr0_sb = [
                constp.tile(
                    [C, WY * (e - b)], F32, name=f"fr0s{s}", tag=f"fr0s{s}"
                )
                for s, (b, e) in enumerate(SLABS)
            ]
            fr12_sb = [
                constp.tile([C, WY * PW], F32, name=f"fr{r}sb", tag=f"fr{r}sb")
                for r in (1, 2)
            ]
            mask_sb = constp.tile([128, WIN], BF16, tag="mask")
            ident_sb = constp.tile([128, 128], BF16, tag="ident")

            nc.sync.dma_start(out=ft_sb[:, 0:256], in_=ft_d.ap()[:, 0:256])
            nc.sync.dma_start(out=fr0_sb[0][:], in_=fr0_d[0].ap())
            # mask + ident ride the parallel scalar HWDGE queue (tiny)
            nc.scalar.dma_start(out=mask_sb[:], in_=mask_d.ap())
            nc.scalar.dma_start(out=ident_sb[:], in_=ident_d.ap())
            nc.sync.dma_start(out=ft_sb[:, 256:], in_=ft_d.ap()[:, 256:])
            nc.sync.dma_start(out=fr0_sb[1][:], in_=fr0_d[1].ap())
            nc.sync.dma_start(out=fr0_sb[2][:], in_=fr0_d[2].ap())
            nc.sync.dma_start(out=fr12_sb[0][:], in_=fr12_d[0].ap())
            nc.sync.dma_start(out=fr12_sb[1][:], in_=fr12_d[1].ap())

            # PE warm-up: ~3us of dummy matmuls on a memset tile right after
            # the preamble so the HAM clock-gate reaches 2.4 GHz before the
            # first real matmul (outputs are never read)
            warm_sb = smallp.tile([128, 128], BF16, tag="warm_sb")
            nc.vector.memset(warm_sb[:], 0.0)
            wps = psump.tile([128, 1024], F32, tag="ps")
            for w in range(30):
                nc.tensor.matmul(
                    wps[:, 0:128], warm_sb[:], warm_sb[:],
                    start=True, stop=True,
                )

            max8 = smallp.tile([128, NRT * 8], F32, tag="max8")
            idx = smallp.tile([128, NRT * 8], U32, tag="idx")
            gath = smallp.tile([128, NRT * CQ], F32, tag="gath")
            gathv = gath[:].rearrange("p (s c) -> p s c", c=CQ)
            es = smallp.tile([128, NRT], F32, tag="es")
            den = smallp.tile([128, NT], F32, tag="den")
            rec = smallp.tile([128, NT], F32, tag="rec")
            num = smallp.tile([128, NT * CQ], F32, tag="num")
            numv = num[:].rearrange("p (s c) -> p s c", c=CQ)

            for r in range(NREF):
                for t in range(NT):
                    rt = r * NT + t

                    # ---- corr Gram + additive mask into two PSUM banks ----
                    ps = psump.tile([128, 1024], F32, tag="ps")
                    lhsT = ft_sb[:, t * 128 : (t + 1) * 128]
                    if r == 0:
                        s = SLAB_OF_TILE[t]
                        b0, e0 = SLABS[s]
                        frv = fr0_sb[s][:].rearrange(
                            "c (y x) -> c y x", x=e0 - b0
                        )
                        off = t * XB - b0
                    else:
                        frv = fr12_sb[r - 1][:].rearrange(
                            "c (y x) -> c y x", x=PW
                        )
                        off = t * XB
                    rhs1 = frv[:, 0:HALF, off : off + WX]
                    rhs2 = frv[:, HALF:WY, off : off + WX]
                    nc.tensor.matmul(
                        ps[:, 0:NHALF], lhsT, rhs1, start=True, stop=False
                    )
                    nc.tensor.matmul(
                        ps[:, 512 : 512 + NHALF], lhsT, rhs2, start=True,
                        stop=False,
                    )
                    nc.tensor.matmul(
                        ps[:, 0:NHALF], ident_sb[:], mask_sb[:, 0:NHALF],
                        start=False, stop=True,
                    )
                    nc.tensor.matmul(
                        ps[:, 512 : 512 + NHALF], ident_sb[:],
                        mask_sb[:, NHALF:WIN], start=False, stop=True,
                    )
                    psv = ps[:].rearrange("p (b n) -> p b n", b=2)[:, :, 0:NHALF]

                    # ---- ScalarE: masked PSUM -> flat SBUF (frees PSUM) ----
                    tb = tbufp.tile([128, WIN], F32, tag="tb")
                    nc.scalar.activation(
                        out=tb[:].rearrange("p (b n) -> p b n", b=2),
                        in_=psv,
                        func=mybir.ActivationFunctionType.Copy,
                    )

                    # ---- DVE: per-pixel masked max off PSUM (parallel with
                    # the ScalarE copy), argmax index off the SBUF copy ----
                    nc.vector.max(
                        out=max8[:, rt * 8 : (rt + 1) * 8], in_=psv
                    )
                    nc.vector.max_index(
                        idx[:, rt * 8 : (rt + 1) * 8],
                        max8[:, rt * 8 : (rt + 1) * 8],
                        tb[:],
                    )

                    # ---- per-tile gather of the argmax pixel (3ch) ----
                    nc.gpsimd.indirect_dma_start(
                        out=gathv[:, rt],
                        out_offset=None,
                        in_=qt_d[rt].ap(),
                        in_offset=IndirectOffsetOnAxis(
                            ap=idx[:, rt * 8 : rt * 8 + 1], axis=0
                        ),
                    )

                # ---- per-ref softmax numerator/denominator accumulation ----
                # (no max-subtraction: |q_val| < 90 so exp() is safe in fp32)
                qv = max8[
                    :, r * NT * 8 : (r + 1) * NT * 8
                ].rearrange("p (k e) -> p k e", e=8)[:, :, 0]
                e_r = es[:, r * NT : (r + 1) * NT]
                nc.scalar.activation(
                    out=e_r, in_=qv, func=mybir.ActivationFunctionType.Exp
                )
                if r == 0:
                    nc.vector.tensor_copy(out=den[:], in_=e_r)
                else:
                    nc.vector.tensor_tensor(
                        out=den[:], in0=den[:], in1=e_r,
                        op=mybir.AluOpType.add,
                    )
                eb = e_r.rearrange("p (s o) -> p s o", o=1).to_broadcast(
                    [128, NT, CQ]
                )
                gvr = gathv[:, r * NT : (r + 1) * NT]
                if r == 0:
                    nc.vector.tensor_tensor(
                        out=numv, in0=gvr, in1=eb, op=mybir.AluOpType.mult
                    )
                else:
                    term = smallp.tile([128, NT * CQ], F32, tag=f"term{r}")
                    termv = term[:].rearrange("p (s c) -> p s c", c=CQ)
                    # split so only the last 4 tiles' chunk trails the final
                    # gather (the rest overlaps earlier gathers)
                    chunks = ((0, 12), (12, NT)) if r == NREF - 1 else ((0, NT),)
                    for c0, c1 in chunks:
                        nc.vector.tensor_tensor(
                            out=termv[:, c0:c1],
                            in0=gvr[:, c0:c1],
                            in1=eb[:, c0:c1],
                            op=mybir.AluOpType.mult,
                        )
                        nc.vector.tensor_tensor(
                            out=numv[:, c0:c1],
                            in0=numv[:, c0:c1],
                            in1=termv[:, c0:c1],
                            op=mybir.AluOpType.add,
                        )
                if r == NREF - 1:
                    nc.vector.reciprocal(out=rec[:], in_=den[:])
                    rb = rec[:].rearrange("p (s o) -> p s o", o=1).to_broadcast(
                        [128, NT, CQ]
                    )
                    oacc = smallp.tile([128, NT * CQ], F32, tag="oacc")
                    oaccv = oacc[:].rearrange("p (s c) -> p s c", c=CQ)
                    nc.vector.tensor_tensor(
                        out=oaccv, in0=numv, in1=rb, op=mybir.AluOpType.mult
                    )
                    nc.sync.dma_start(out=out_d.ap(), in_=oacc[:])

    nc.compile()
    return nc


def _host_prep(feats_r, feats_t, quantized_r):
    """Build the 8 per-core input maps."""
    frp_full = np.zeros((NREF, C, H + 2 * RAD, PW), np.float32)
    frp_full[:, :, RAD : RAD + H, RAD : RAD + W] = feats_r[:, 0]

    qr = np.ascontiguousarray(quantized_r[:, 0, :, ::SUB, ::SUB], np.float32)
    qrp_full = np.zeros((NREF, H + 2 * RAD, PW, CQ), np.float32)
    qrp_full[:, RAD : RAD + H, RAD : RAD + W, :] = qr.transpose(0, 2, 3, 1)

    # mask[p=(yl,xl), n=(y',x')] = 0 inside pixel (yl,xl)'s own 13x13 patch
    yl = np.arange(ROWS)[:, None, None, None]
    xl = np.arange(XB)[None, :, None, None]
    yw = np.arange(WY)[None, None, :, None]
    xw = np.arange(WX)[None, None, None, :]
    valid = (
        (yw - yl >= 0) & (yw - yl < PS) & (xw - xl >= 0) & (xw - xl < PS)
    )
    import ml_dtypes

    mask = np.where(valid, 0.0, NEG).astype(ml_dtypes.bfloat16).reshape(128, WIN)
    ident = np.eye(128, dtype=np.float32).astype(ml_dtypes.bfloat16)

    in_maps = []
    for k in range(NCORES):
        y0 = ROWS * k
        # [c, yl, t, xl] -> [c, t, yl, xl]: tile-major, pixels contiguous
        ft_core = np.ascontiguousarray(
            feats_t[0][:, y0 : y0 + ROWS, :]
            .reshape(C, ROWS, NT, XB)
            .transpose(0, 2, 1, 3)
            .reshape(C, ROWS * W)
        )
        frp_core = frp_full[:, :, y0 : y0 + WY, :]  # [NREF, C, WY, PW]
        m = {"ft": ft_core, "mask": mask, "ident": ident}
        for s, (b, e) in enumerate(SLABS):
            m[f"fr0s{s}"] = np.ascontiguousarray(
                frp_core[0, :, :, b:e].reshape(C, WY * (e - b))
            )
        for r in (1, 2):
            m[f"fr{r}"] = np.ascontiguousarray(
                frp_core[r].reshape(C, WY * PW)
            )
        qc = qrp_full[:, y0 : y0 + WY, :, :]  # [NREF, WY, PW, CQ]

        def tile_table(rt):
            r, t = divmod(rt, NT)
            return qc[r, :, t * XB : t * XB + WX, :].reshape(WIN, CQ)

        for rt in range(NRT):
            m[f"qt{rt}"] = np.ascontiguousarray(tile_table(rt))
        in_maps.append(m)
    return in_maps


def _install_ntff_shim():
    """This container's antenv lacks axon_hooks, so run_bass_kernel_spmd's
    trace path can't find the NTFF profile hook. Inject the module and
    register the ctypes-based hook from the boot script. Best-effort."""
    try:
        import sys
        import types

        if "antenv.axon_hooks" in sys.modules:
            return
        mod = types.ModuleType("antenv.axon_hooks")
        holder = [None]
        mod.set_axon_ntff_profile_hook = lambda h: holder.__setitem__(0, h)
        mod.get_axon_ntff_profile_hook = lambda: holder[0]
        sys.modules["antenv.axon_hooks"] = mod
        import antenv

        antenv.axon_hooks = mod
        from trn_agent_boot.trn_boot import _ntff_profile_via_ctypes

        hook = _ntff_profile_via_ctypes("/opt/axon/libaxon_pjrt.so")
        if hook is not None:
            mod.set_axon_ntff_profile_hook(hook)
    except Exception as e:  # pragma: no cover - tracing is best-effort
        print(f"ntff shim install failed: {e}")


last_exec_time_ns = None


def kernel(feats_r, feats_t, quantized_r, ref_index=None, current_ind=None):
    global last_exec_time_ns
    feats_r = np.asarray(feats_r, np.float32)
    feats_t = np.asarray(feats_t, np.float32)
    quantized_r = np.asarray(quantized_r, np.float32)

    in_maps = _host_prep(feats_r, feats_t, quantized_r)

    if "nc" not in _CACHE:
        _CACHE["nc"] = _build_program()
    nc = _CACHE["nc"]

    trace = bool(int(os.environ.get("KERNEL_TRACE", "0")))
    kwargs = {}
    if trace:
        _install_ntff_shim()
        tdir = os.environ.get("KERNEL_TRACE_DIR")
        if tdir:
            os.makedirs(tdir, exist_ok=True)
            kwargs["tmpdir"] = tdir
    res = run_bass_kernel_spmd(
        nc, in_maps, list(range(NCORES)), trace=trace, **kwargs
    )
    last_exec_time_ns = res.exec_time_ns

    out = np.concatenate(
        [_unshard_core(res.results[k]["out"]) for k in range(NCORES)], axis=1
    )
    return np.ascontiguousarray(out.reshape(1, CQ, H, W), np.float32)


def _unshard_core(raw):
    # raw [128, NT*CQ] with partition p=(yl,xl), free (t, c) -> [CQ, ROWS, W]
    r = np.asarray(raw).reshape(ROWS, XB, NT, CQ)
    return r.transpose(3, 0, 2, 1).reshape(CQ, ROWS, W)


# revision 33
# speedup vs baseline: 1.0591x; 1.0591x over previous
"""Trainium2 Bass kernel for nn_Colorizer (retrieval_knn).

Computation (per reference frame r of 3, for each pixel p of a 128x128 image):
  corr[r, n, p] = <feats_t[:, p], feats_r[r, :, p + offset(n)]>   n in 13x13 window
  q_val[r, p]  = max_n corr ; q_idx[r, p] = argmax_n corr (first occurrence)
  gathered[r, c, p] = quantized_sub[r, c, p + offset(q_idx)]      (zero padded)
  out[c, p] = sum_r softmax_r(q_val)[r] * gathered[r, c, p]

Sharding: the spatial h dim is split into 8 bands of 16 rows (one per core);
each core handles all 3 refs for its band, so the softmax over refs is local
and no device collective is needed.  Host reassembles the row bands.

Device algorithm per core, per tile of 128 pixels (16 rows x 8 cols):
  - TensorE: Gram matrix between the tile's feats_t vectors (lhsT, c=128
    contraction) and the 28x20 zero-padded feats_r halo window (560 cols ->
    2 PSUM banks), then a constant additive -1e30 valid-window mask via an
    identity matmul accumulate.
  - ScalarE copies the masked PSUM to a flat SBUF tile buffer (fast PSUM
    turnaround for the matmul pipeline; SBUF reads are cheaper for the DVE).
  - DVE MAX8 + FIND_INDEX8 on the SBUF copy -> per-pixel masked max (q_val)
    and argmax index.  Per-tile (not batched) so the indirect gathers spread
    evenly over the whole kernel: each gather op costs a fixed ~1.1us of
    GPSIMD descriptor-generation time and the gather queue is the scarcest
    resource.
  - per-tile indirect DMA gathers the argmax pixel (3ch) from a per-tile
    [560, 3] DRAM table indexed by the raw window index.
  - softmax over the 3 refs without max-subtraction (corr maxima are < 90 so
    exp() cannot overflow), accumulated incrementally per ref so only a ~1us
    chain remains after the last gather.

Input DMA is priority-ordered on the sync HWDGE queue (first tiles' feats
first; feats_r of ref 0 is shipped as 3 overlapping contiguous column slabs
so the first matmul can start early), with the mask + identity on the
parallel scalar HWDGE queue.
"""

import os

import numpy as np

import concourse.bass as bass
import concourse.mybir as mybir
import concourse.tile as tile
from concourse import bacc
from concourse.bass import IndirectOffsetOnAxis
from concourse.bass_utils import run_bass_kernel_spmd

F32 = mybir.dt.float32
BF16 = mybir.dt.bfloat16
U32 = mybir.dt.uint32

NCORES = 8
NREF, C, H, W = 3, 128, 128, 128
RAD = 6                      # patch radius
PS = 2 * RAD + 1             # 13
CQ = 3                       # quantized channels
SUB = 4                      # quantized_r spatial subsample stride
ROWS = H // NCORES           # 16 rows per core
XB = 8                       # x block size
NT = W // XB                 # 16 tiles per ref
WY = ROWS + 2 * RAD          # 28 window rows
WX = XB + 2 * RAD            # 20 window cols
WIN = WY * WX                # 560
HALF = WY // 2               # 14 window rows per PSUM bank
NHALF = HALF * WX            # 280 columns per matmul
PW = W + 2 * RAD             # 140 padded width
NRT = NREF * NT              # 48 (ref, tile) pairs
NEG = -1.0e30

# ref-0 feats_r is shipped as 3 contiguous, overlapping column slabs so the
# first tiles' windows arrive early: tile t needs padded cols [8t, 8t+20).
SLABS = [(0, 28), (16, 76), (64, 140)]          # tiles 0-1 / 2-7 / 8-15
SLAB_OF_TILE = [0] * 2 + [1] * 6 + [2] * 8

_CACHE: dict = {}


def _max_index_raw(nc, out, in_max, in_values):
    """max_index accepting multi-dim / broadcast APs (e.g. duplicated needle
    views); the bass wrapper's 2-D asserts are stricter than the hardware."""
    eng = nc.vector
    return eng.add_instruction(
        mybir.InstMaxIndex(
            name=nc.get_next_instruction_name(),
            ins=[eng.lower_ap(in_max), eng.lower_ap(in_values)],
            outs=[eng.lower_ap(out)],
        )
    )


def _build_program() -> bacc.Bacc:
    nc = bacc.Bacc("TRN2", target_bir_lowering=False, debug=False)

    ft_d = nc.dram_tensor("ft", [C, ROWS * W], F32, kind="ExternalInput")
    fr0_d = [
        nc.dram_tensor(f"fr0s{s}", [C, WY * (e - b)], F32, kind="ExternalInput")
        for s, (b, e) in enumerate(SLABS)
    ]
    fr12_d = [
        nc.dram_tensor(f"fr{r}", [C, WY * PW], F32, kind="ExternalInput")
        for r in (1, 2)
    ]
    mask_d = nc.dram_tensor("mask", [128, WIN], BF16, kind="ExternalInput")
    ident_d = nc.dram_tensor("ident", [128, 128], BF16, kind="ExternalInput")
    # one [4*WIN, CQ] gather table per quad of tiles: the raw quad-local
    # argmax index is the gather row, no on-device index arithmetic needed
    qt_d = [
        nc.dram_tensor(f"qt{rt}", [WIN, CQ], F32, kind="ExternalInput")
        for rt in range(NRT)
    ]
    # raw layout [pixel_partition=(yl,xl), tile, channel]; host untangles
    out_d = nc.dram_tensor("out", [128, NT * CQ], F32, kind="ExternalOutput")

    with tile.TileContext(nc) as tc:
        with (
            tc.tile_pool(name="const", bufs=1) as constp,
            tc.tile_pool(name="psum", bufs=4, space="PSUM") as psump,
            tc.tile_pool(name="tbuf", bufs=4) as tbufp,
            tc.tile_pool(name="small", bufs=1) as smallp,
        ):
            # ---- input DMA, priority order on the sync HWDGE queue ----
            ft_sb = constp.tile([C, ROWS * W], F32, tag="ft")
            fr0_sb = [
                constp.tile(
                    [C, WY * (e - b)], F32, name=f"fr0s{s}", tag=f"fr0s{s}"
                )
                for s, (b, e) in enumerate(SLABS)
            ]
            fr12_sb = [
                constp.tile([C, WY * PW], F32, name=f"fr{r}sb", tag=f"fr{r}sb")
                for r in (1, 2)
            ]
            mask_sb = constp.tile([128, WIN], BF16, tag="mask")
            ident_sb = constp.tile([128, 128], BF16, tag="ident")

            nc.sync.dma_start(out=ft_sb[:, 0:256], in_=ft_d.ap()[:, 0:256])
            nc.sync.dma_start(out=fr0_sb[0][:], in_=fr0_d[0].ap())
            # mask + ident ride the parallel scalar HWDGE queue (tiny)
            nc.scalar.dma_start(out=mask_sb[:], in_=mask_d.ap())
            nc.scalar.dma_start(out=ident_sb[:], in_=ident_d.ap())
            nc.sync.dma_start(out=ft_sb[:, 256:], in_=ft_d.ap()[:, 256:])
            nc.sync.dma_start(out=fr0_sb[1][:], in_=fr0_d[1].ap())
            nc.sync.dma_start(out=fr0_sb[2][:], in_=fr0_d[2].ap())
            nc.sync.dma_start(out=fr12_sb[0][:], in_=fr12_d[0].ap())
            nc.sync.dma_start(out=fr12_sb[1][:], in_=fr12_d[1].ap())

            max8 = smallp.tile([128, NRT * 8], F32, tag="max8")
            idx = smallp.tile([128, NRT * 8], U32, tag="idx")
            gath = smallp.tile([128, NRT * CQ], F32, tag="gath")
            gathv = gath[:].rearrange("p (s c) -> p s c", c=CQ)
            es = smallp.tile([128, NRT], F32, tag="es")
            den = smallp.tile([128, NT], F32, tag="den")
            rec = smallp.tile([128, NT], F32, tag="rec")
            num = smallp.tile([128, NT * CQ], F32, tag="num")
            numv = num[:].rearrange("p (s c) -> p s c", c=CQ)

            for r in range(NREF):
                for t in range(NT):
                    rt = r * NT + t

                    # ---- corr Gram + additive mask into two PSUM banks ----
                    ps = psump.tile([128, 1024], F32, tag="ps")
                    lhsT = ft_sb[:, t * 128 : (t + 1) * 128]
                    if r == 0:
                        s = SLAB_OF_TILE[t]
                        b0, e0 = SLABS[s]
                        frv = fr0_sb[s][:].rearrange(
                            "c (y x) -> c y x", x=e0 - b0
                        )
                        off = t * XB - b0
                    else:
                        frv = fr12_sb[r - 1][:].rearrange(
                            "c (y x) -> c y x", x=PW
                        )
                        off = t * XB
                    rhs1 = frv[:, 0:HALF, off : off + WX]
                    rhs2 = frv[:, HALF:WY, off : off + WX]
                    nc.tensor.matmul(
                        ps[:, 0:NHALF], lhsT, rhs1, start=True, stop=False
                    )
                    nc.tensor.matmul(
                        ps[:, 512 : 512 + NHALF], lhsT, rhs2, start=True,
                        stop=False,
                    )
                    nc.tensor.matmul(
                        ps[:, 0:NHALF], ident_sb[:], mask_sb[:, 0:NHALF],
                        start=False, stop=True,
                    )
                    nc.tensor.matmul(
                        ps[:, 512 : 512 + NHALF], ident_sb[:],
                        mask_sb[:, NHALF:WIN], start=False, stop=True,
                    )
                    psv = ps[:].rearrange("p (b n) -> p b n", b=2)[:, :, 0:NHALF]

                    # ---- ScalarE: masked PSUM -> flat SBUF (frees PSUM) ----
                    tb = tbufp.tile([128, WIN], F32, tag="tb")
                    nc.scalar.activation(
                        out=tb[:].rearrange("p (b n) -> p b n", b=2),
                        in_=psv,
                        func=mybir.ActivationFunctionType.Copy,
                    )

                    # ---- DVE: per-pixel masked max off PSUM (parallel with
                    # the ScalarE copy), argmax index off the SBUF copy ----
                    nc.vector.max(
                        out=max8[:, rt * 8 : (rt + 1) * 8], in_=psv
                    )
                    nc.vector.max_index(
                        idx[:, rt * 8 : (rt + 1) * 8],
                        max8[:, rt * 8 : (rt + 1) * 8],
                        tb[:],
                    )

                    # ---- per-tile gather of the argmax pixel (3ch) ----
                    nc.gpsimd.indirect_dma_start(
                        out=gathv[:, rt],
                        out_offset=None,
                        in_=qt_d[rt].ap(),
                        in_offset=IndirectOffsetOnAxis(
                            ap=idx[:, rt * 8 : rt * 8 + 1], axis=0
                        ),
                    )

                # ---- per-ref softmax numerator/denominator accumulation ----
                # (no max-subtraction: |q_val| < 90 so exp() is safe in fp32)
                qv = max8[
                    :, r * NT * 8 : (r + 1) * NT * 8
                ].rearrange("p (k e) -> p k e", e=8)[:, :, 0]
                e_r = es[:, r * NT : (r + 1) * NT]
                nc.scalar.activation(
                    out=e_r, in_=qv, func=mybir.ActivationFunctionType.Exp
                )
                if r == 0:
                    nc.vector.tensor_copy(out=den[:], in_=e_r)
                else:
                    nc.vector.tensor_tensor(
                        out=den[:], in0=den[:], in1=e_r,
                        op=mybir.AluOpType.add,
                    )
                eb = e_r.rearrange("p (s o) -> p s o", o=1).to_broadcast(
                    [128, NT, CQ]
                )
                gvr = gathv[:, r * NT : (r + 1) * NT]
                if r == 0:
                    nc.vector.tensor_tensor(
                        out=numv, in0=gvr, in1=eb, op=mybir.AluOpType.mult
                    )
                else:
                    term = smallp.tile([128, NT * CQ], F32, tag=f"term{r}")
                    termv = term[:].rearrange("p (s c) -> p s c", c=CQ)
                    # split so only the last 4 tiles' chunk trails the final
                    # gather (the rest overlaps earlier gathers)
                    chunks = ((0, 12), (12, NT)) if r == NREF - 1 else ((0, NT),)
                    for c0, c1 in chunks:
                        nc.vector.tensor_tensor(
                            out=termv[:, c0:c1],
                            in0=gvr[:, c0:c1],
                            in1=eb[:, c0:c1],
                            op=mybir.AluOpType.mult,
                        )
                        nc.vector.tensor_tensor(
                            out=numv[:, c0:c1],
                            in0=numv[:, c0:c1],
                            in1=termv[:, c0:c1],
                            op=mybir.AluOpType.add,
                        )
                if r == NREF - 1:
                    nc.vector.reciprocal(out=rec[:], in_=den[:])
                    rb = rec[:].rearrange("p (s o) -> p s o", o=1).to_broadcast(
                        [128, NT, CQ]
                    )
                    oacc = smallp.tile([128, NT * CQ], F32, tag="oacc")
                    oaccv = oacc[:].rearrange("p (s c) -> p s c", c=CQ)
                    nc.vector.tensor_tensor(
                        out=oaccv, in0=numv, in1=rb, op=mybir.AluOpType.mult
                    )
                    nc.sync.dma_start(out=out_d.ap(), in_=oacc[:])

    nc.compile()
    return nc


def _host_prep(feats_r, feats_t, quantized_r):
    """Build the 8 per-core input maps."""
    frp_full = np.zeros((NREF, C, H + 2 * RAD, PW), np.float32)
    frp_full[:, :, RAD : RAD + H, RAD : RAD + W] = feats_r[:, 0]

    qr = np.ascontiguousarray(quantized_r[:, 0, :, ::SUB, ::SUB], np.float32)
    qrp_full = np.zeros((NREF, H + 2 * RAD, PW, CQ), np.float32)
    qrp_full[:, RAD : RAD + H, RAD : RAD + W, :] = qr.transpose(0, 2, 3, 1)

    # mask[p=(yl,xl), n=(y',x')] = 0 inside pixel (yl,xl)'s own 13x13 patch
    yl = np.arange(ROWS)[:, None, None, None]
    xl = np.arange(XB)[None, :, None, None]
    yw = np.arange(WY)[None, None, :, None]
    xw = np.arange(WX)[None, None, None, :]
    valid = (
        (yw - yl >= 0) & (yw - yl < PS) & (xw - xl >= 0) & (xw - xl < PS)
    )
    import ml_dtypes

    mask = np.where(valid, 0.0, NEG).astype(ml_dtypes.bfloat16).reshape(128, WIN)
    ident = np.eye(128, dtype=np.float32).astype(ml_dtypes.bfloat16)

    in_maps = []
    for k in range(NCORES):
        y0 = ROWS * k
        # [c, yl, t, xl] -> [c, t, yl, xl]: tile-major, pixels contiguous
        ft_core = np.ascontiguousarray(
            feats_t[0][:, y0 : y0 + ROWS, :]
            .reshape(C, ROWS, NT, XB)
            .transpose(0, 2, 1, 3)
            .reshape(C, ROWS * W)
        )
        frp_core = frp_full[:, :, y0 : y0 + WY, :]  # [NREF, C, WY, PW]
        m = {"ft": ft_core, "mask": mask, "ident": ident}
        for s, (b, e) in enumerate(SLABS):
            m[f"fr0s{s}"] = np.ascontiguousarray(
                frp_core[0, :, :, b:e].reshape(C, WY * (e - b))
            )
        for r in (1, 2):
            m[f"fr{r}"] = np.ascontiguousarray(
                frp_core[r].reshape(C, WY * PW)
            )
        qc = qrp_full[:, y0 : y0 + WY, :, :]  # [NREF, WY, PW, CQ]

        def tile_table(rt):
            r, t = divmod(rt, NT)
            return qc[r, :, t * XB : t * XB + WX, :].reshape(WIN, CQ)

        for rt in range(NRT):
            m[f"qt{rt}"] = np.ascontiguousarray(tile_table(rt))
        in_maps.append(m)
    return in_maps


def _install_ntff_shim():
    """This container's antenv lacks axon_hooks, so run_bass_kernel_spmd's
    trace path can't find the NTFF profile hook. Inject the module and
    register the ctypes-based hook from the boot script. Best-effort."""
    try:
        import sys
        import types

        if "antenv.axon_hooks" in sys.modules:
            return
        mod = types.ModuleType("antenv.axon_hooks")
        holder = [None]
        mod.set_axon_ntff_profile_hook = lambda h: holder.__setitem__(0, h)
        mod.get_axon_ntff_profile_hook = lambda: holder[0]
        sys.modules["antenv.axon_hooks"] = mod
        import antenv

        antenv.axon_hooks = mod
        from trn_agent_boot.trn_boot import _ntff_profile_via_ctypes

        hook = _ntff_profile_via_ctypes("/opt/axon/libaxon_pjrt.so")
        if hook is not None:
            mod.set_axon_ntff_profile_hook(hook)
    except Exception as e:  # pragma: no cover - tracing is best-effort
        print(f"ntff shim install failed: {e}")


last_exec_time_ns = None


def kernel(feats_r, feats_t, quantized_r, ref_index=None, current_ind=None):
    global last_exec_time_ns
    feats_r = np.asarray(feats_r, np.float32)
    feats_t = np.asarray(feats_t, np.float32)
    quantized_r = np.asarray(quantized_r, np.float32)

    in_maps = _host_prep(feats_r, feats_t, quantized_r)

    if "nc" not in _CACHE:
        _CACHE["nc"] = _build_program()
    nc = _CACHE["nc"]

    trace = bool(int(os.environ.get("KERNEL_TRACE", "0")))
    kwargs = {}
    if trace:
        _install_ntff_shim()
        tdir = os.environ.get("KERNEL_TRACE_DIR")
        if tdir:
            os.makedirs(tdir, exist_ok=True)
            kwargs["tmpdir"] = tdir
    res = run_bass_kernel_spmd(
        nc, in_maps, list(range(NCORES)), trace=trace, **kwargs
    )
    last_exec_time_ns = res.exec_time_ns

    out = np.concatenate(
        [_unshard_core(res.results[k]["out"]) for k in range(NCORES)], axis=1
    )
    return np.ascontiguousarray(out.reshape(1, CQ, H, W), np.float32)


def _unshard_core(raw):
    # raw [128, NT*CQ] with partition p=(yl,xl), free (t, c) -> [CQ, ROWS, W]
    r = np.asarray(raw).reshape(ROWS, XB, NT, CQ)
    return r.transpose(3, 0, 2, 1).reshape(CQ, ROWS, W)
